# revision 3
# baseline (speedup 1.0000x reference)
"""Causal flash attention (B=2, H=16, S=2048, D=64, fp32) on 8 TRN2 NeuronCores.

Strategy: shard batch*heads (32) across 8 cores -> 4 heads/core. Per head,
compute transposed scores S^T[k, q] = K Q^T via PE (fp16 inputs, fp32 PSUM
accumulate), then exponentiate. The exp pass (the kernel bottleneck: every
score element must cross PSUM->SBUF through ScalarE or VectorE at 1
elem/lane/cycle) is SPLIT between the two engines:
  - ScalarE ACT computes exact exp (softmax scale folded into the activation
    input scale, fp16 out).
  - VectorE computes a Schraudolph-style exp2 bit-trick: i16 = rne(s*A + B)
    with A = 1024*SCALE*log2(e), bitcast to fp16 gives 2^(t)*(1+eps),
    |eps| <= 3% zero-mean. Softmax normalization cancels the row-mean
    component; the residual error lands well under the 2e-2 budget.
Off-diagonal score groups are routed between the engines by a ratio chosen
to balance their busy time. Diagonal groups go through ACT with strided APs
that skip the fully-invalid left half of the second diagonal tile, then get
a multiplicative 0/1 causal mask (batched across the two packed heads) on
VectorE. PV via PE with a ones column appended to V so the softmax
denominator falls out of the same matmul. The output leaves the device
transposed ([d+1, q] per head, fp32); the host divides by the denominator
row and transposes back.

Two heads are packed into the 128 SBUF partitions (d=64 each) so QK matmuls
for a head pair run concurrently on disjoint PE row groups. B-diagonal
tiles (fully masked left half) are restricted to their valid right half in
QK, exp, and PV (N=128). Inputs stream in over three DMA queues (k->sync,
q->tensor, v->gpsimd) with v host-prepacked to give each SBUF partition one
contiguous HBM run.
"""

import numpy as np

B, H, S, D = 2, 16, 2048, 64
BH = B * H
NCORES = 8
HPC = BH // NCORES  # heads per core
SCALE = 0.125
W = 256             # q-block width (matmul moving dim)
TK = 128            # k-tile height
NKT = S // TK       # 16 k-tiles
NQB = S // W        # 8 q-blocks
G = 2               # k-tiles per exp group; [128, 2*G*W] fp32 = 2 PSUM banks

LOG2E = 1.4426950408889634
A_C = 1024.0 * SCALE * LOG2E      # Schraudolph scale
B_C = 15360.0 - 58.7              # (15<<10) + geometric-mean centering
DVE_FRAC = 0.40                   # share of off-diag exp groups on VectorE

_CACHE = {}


def _build_nc():
    import concourse.bass as bass  # noqa: F401
    import concourse.mybir as mybir
    import concourse.tile as tile
    from concourse import bacc

    f32 = mybir.dt.float32
    f16 = mybir.dt.float16
    i16 = mybir.dt.int16
    EXP = mybir.ActivationFunctionType.Exp
    MULT = mybir.AluOpType.mult
    ADD = mybir.AluOpType.add

    nc = bacc.Bacc("TRN2", target_bir_lowering=False, debug=False, num_devices=NCORES)

    qt_d = nc.dram_tensor("qt", [HPC, D, S], f16, kind="ExternalInput").ap()
    kt_d = nc.dram_tensor("kt", [HPC, D, S], f16, kind="ExternalInput").ap()
    # v arrives host-packed as [HPC, 128, NKT*(D+1)]: partition-major so each
    # SBUF partition reads one contiguous run; ones column pre-appended.
    v_d = nc.dram_tensor("v", [HPC, 128, NKT * (D + 1)], f16, kind="ExternalInput").ap()
    o_d = nc.dram_tensor("outT", [HPC, D + 1, S], f32, kind="ExternalOutput").ap()

    with tile.TileContext(nc) as tc:
        const_pool = tc.alloc_tile_pool(name="const", bufs=1)
        kq_pool = tc.alloc_tile_pool(name="kq", bufs=1)
        vx_pool = tc.alloc_tile_pool(name="vx", bufs=1)
        p_pool = tc.alloc_tile_pool(name="p", bufs=12)
        o_pool = tc.alloc_tile_pool(name="o", bufs=8)
        ps_pool = tc.alloc_tile_pool(name="ps", bufs=3, space="PSUM")
        pv_pool = tc.alloc_tile_pool(name="pv", bufs=2, space="PSUM")

        # Multiplicative causal mask for A-diagonal tiles: maskA[x, y] = 1 if
        # y >= x else 0. B-diagonal right halves reuse maskA[:, 0:128].
        maskA = const_pool.tile([128, W], f16, tag="maskA")
        nc.gpsimd.memset(maskA[:], 1.0)
        nc.gpsimd.affine_select(
            out=maskA[:], in_=maskA[:],
            compare_op=mybir.AluOpType.is_ge,
            fill=0.0, base=0,
            pattern=[[1, W]], channel_multiplier=-1,
        )
        maskA_bc = maskA[:].unsqueeze(1).broadcast_to([128, 2, W])
        maskAL_bc = maskA[:, 0:128].unsqueeze(1).broadcast_to([128, 2, 128])

        # Input loads. kt/qt are packed 2 heads per 128 partitions, chunked so
        # the pieces the first q-blocks need (descending qb order: low k-tiles,
        # high q columns) arrive first. k on the sync DMA queue, q on the
        # tensor queue, v on the gpsimd queue.
        ktc = {}
        qtc = {}
        vxc = {}
        for pr in range(2):
            hA, hB = 2 * pr, 2 * pr + 1
            hsl = slice(2 * pr, 2 * pr + 2)
            kchunk = kq_pool.tile([128, S], f16, tag=f"ktc{pr}", name=f"ktc{pr}")
            qchunk = kq_pool.tile([128, S], f16, tag=f"qtc{pr}", name=f"qtc{pr}")
            ktc[pr] = kchunk
            qtc[pr] = qchunk
            for h in (hA, hB):
                vchunk = vx_pool.tile([128, NKT * (D + 1)], f16, tag=f"vx{h}",
                                      name=f"vx{h}")
                nc.gpsimd.dma_start(vchunk[:], v_d[h])
                vxc[h] = vchunk
        for pr in range(2):
            hsl = slice(2 * pr, 2 * pr + 2)
            for ks in (slice(0, 512), slice(512, 1024), slice(1024, 1536),
                       slice(1536, S)):
                nc.sync.dma_start(
                    ktc[pr][:, ks],
                    kt_d[hsl, :, ks].rearrange("h d s -> (h d) s"),
                )
            for qs in (slice(1536, S), slice(1024, 1536), slice(512, 1024),
                       slice(0, 512)):
                nc.scalar.dma_start(
                    qtc[pr][:, qs],
                    qt_d[hsl, :, qs].rearrange("h d s -> (h d) s"),
                )

        def ktile(pr, kt):
            return ktc[pr][:, kt * TK:(kt + 1) * TK]

        def vx(h, kt):
            return vxc[h][:, kt * (D + 1):(kt + 1) * (D + 1)]

        route_acc = [0.0]

        def route_dve():
            route_acc[0] += DVE_FRAC
            if route_acc[0] >= 1.0:
                route_acc[0] -= 1.0
                return True
            return False

        out_q = [0]

        def out_dma(dst, src):
            eng = nc.sync if out_q[0] % 2 == 0 else nc.gpsimd
            out_q[0] += 1
            eng.dma_start(dst, src)

        # Main pipeline, one head-pair at a time. Score groups are
        # [128, 2*G*W] (2 PSUM banks), triple-buffered so QK always runs
        # 1-2 groups ahead of the exp that consumes them; PV matmuls lag
        # one group behind the exp. Head A occupies group cols [0, gw*W),
        # head B [gw*W, 2*gw*W).
        for pr in range(2):
            hA, hB = 2 * pr, 2 * pr + 1
            pending = None  # (qb, g0, gw, p, pvA, pvB)

            def flush_pending():
                nonlocal pending
                if pending is None:
                    return
                qb, g0, gw, p, pvA, pvB = pending
                nkt = 2 * qb + 2
                for j in range(gw):
                    kt = g0 + j
                    bdiag = (kt == nkt - 1)
                    for off, vxt, pv in ((0, vx(hA, kt), pvA),
                                         (gw * W, vx(hB, kt), pvB)):
                        if bdiag:
                            nc.tensor.matmul(
                                pv[:, 128:W],
                                vxt,
                                p[:, off + j * W + 128:off + (j + 1) * W],
                                start=False,
                                stop=True,
                                skip_group_check=True,
                            )
                        else:
                            nc.tensor.matmul(
                                pv[:],
                                vxt,
                                p[:, off + j * W:off + (j + 1) * W],
                                start=(kt == 0),
                                stop=False,
                                skip_group_check=True,
                            )
                if g0 + gw == nkt:  # last group of the q-block: write out
                    oA = o_pool.tile([D + 1, W], f32, tag="o")
                    oB = o_pool.tile([D + 1, W], f32, tag="o")
                    nc.vector.tensor_copy(oA[:], pvA[:])
                    nc.vector.tensor_copy(oB[:], pvB[:])
                    out_dma(o_d[hA, :, qb * W:(qb + 1) * W], oA[:])
                    out_dma(o_d[hB, :, qb * W:(qb + 1) * W], oB[:])
                pending = None

            for qb in reversed(range(NQB)):
                nkt = 2 * qb + 2
                pvA = pv_pool.tile([D + 1, W], f32, tag="pv", name="pvA")
                pvB = pv_pool.tile([D + 1, W], f32, tag="pv", name="pvB")
                qA = qtc[pr][0:64, qb * W:(qb + 1) * W]
                qB = qtc[pr][64:128, qb * W:(qb + 1) * W]
                for g0 in range(0, nkt, G):
                    gw = min(G, nkt - g0)  # always 2 (nkt even)
                    diag = (g0 == nkt - 2)
                    sG = ps_pool.tile([128, 2 * G * W], f32, tag="sG")
                    for j in range(gw):
                        kt = g0 + j
                        bdiag = diag and j == 1
                        if bdiag:
                            # fully-masked left half: compute right half only
                            nc.tensor.matmul(
                                sG[:, j * W + 128:(j + 1) * W],
                                ktile(pr, kt)[0:64], qA[:, 128:W],
                                start=True, stop=True,
                            )
                            nc.tensor.matmul(
                                sG[:, gw * W + j * W + 128:gw * W + (j + 1) * W],
                                ktile(pr, kt)[64:128], qB[:, 128:W],
                                start=True, stop=True,
                            )
                        else:
                            nc.tensor.matmul(
                                sG[:, j * W:(j + 1) * W],
                                ktile(pr, kt)[0:64], qA,
                                start=True, stop=True,
                            )
                            nc.tensor.matmul(
                                sG[:, gw * W + j * W:gw * W + (j + 1) * W],
                                ktile(pr, kt)[64:128], qB,
                                start=True, stop=True,
                            )
                    p = p_pool.tile([128, 2 * G * W], f16, tag="p")
                    if not diag:
                        if route_dve():
                            nc.vector.tensor_scalar(
                                p[:, :2 * gw * W].bitcast(i16),
                                sG[:, :2 * gw * W],
                                A_C, B_C, MULT, ADD,
                            )
                        else:
                            nc.scalar.activation(
                                p[:, :2 * gw * W], sG[:, :2 * gw * W], EXP,
                                scale=SCALE,
                            )
                    else:
                        s3 = sG[:].rearrange("c (h x) -> c h x", x=gw * W)
                        p3 = p[:].rearrange("c (h x) -> c h x", x=gw * W)
                        # A-diagonal tiles (cols [0,W) of each head half)
                        nc.scalar.activation(
                            p3[:, :, 0:W], s3[:, :, 0:W], EXP, scale=SCALE
                        )
                        # B-diagonal right halves (cols [W+128, 2W))
                        nc.scalar.activation(
                            p3[:, :, W + 128:2 * W], s3[:, :, W + 128:2 * W],
                            EXP, scale=SCALE,
                        )
                        nc.vector.tensor_mul(
                            p3[:, :, 0:W], p3[:, :, 0:W], maskA_bc
                        )
                        nc.vector.tensor_mul(
                            p3[:, :, W + 128:2 * W], p3[:, :, W + 128:2 * W],
                            maskAL_bc,
                        )
                    flush_pending()
                    pending = (qb, g0, gw, p, pvA, pvB)
            flush_pending()

        pv_pool.release()
        ps_pool.release()
        o_pool.release()
        p_pool.release()
        vx_pool.release()
        kq_pool.release()
        const_pool.release()

    nc.compile()
    return nc


def _get_nc():
    if "nc" not in _CACHE:
        _CACHE["nc"] = _build_nc()
    return _CACHE["nc"]


def _prep_inputs(q, k, v):
    qf = np.ascontiguousarray(np.asarray(q, dtype=np.float32)).reshape(BH, S, D)
    kf = np.ascontiguousarray(np.asarray(k, dtype=np.float32)).reshape(BH, S, D)
    vf = np.ascontiguousarray(np.asarray(v, dtype=np.float32)).reshape(BH, S, D)
    vx = np.empty((BH, S, D + 1), np.float16)
    vx[:, :, :D] = vf
    vx[:, :, D] = 1.0
    # pack v partition-major: [BH, NKT, 128, D+1] -> [BH, 128, NKT*(D+1)]
    vp = np.ascontiguousarray(
        vx.reshape(BH, NKT, 128, D + 1).transpose(0, 2, 1, 3)
    ).reshape(BH, 128, NKT * (D + 1))
    qt = qf.transpose(0, 2, 1).astype(np.float16)
    kt = kf.transpose(0, 2, 1).astype(np.float16)
    in_maps = []
    for c in range(NCORES):
        sl = slice(HPC * c, HPC * (c + 1))
        in_maps.append({
            "qt": np.ascontiguousarray(qt[sl]),
            "kt": np.ascontiguousarray(kt[sl]),
            "v": np.ascontiguousarray(vp[sl]),
        })
    return in_maps


def _postprocess(results):
    out = np.empty((B, H, S, D), np.float32)
    for c in range(NCORES):
        ot = results[c]["outT"]  # [HPC, D+1, S]
        o = (ot[:, :D, :] / ot[:, D:D + 1, :]).transpose(0, 2, 1)  # [HPC, S, D]
        for i in range(HPC):
            bh = HPC * c + i
            out[bh // H, bh % H] = o[i]
    return out


def run(q, k, v, trace=False):
    from concourse.bass_utils import run_bass_kernel_spmd

    nc = _get_nc()
    in_maps = _prep_inputs(q, k, v)
    res = run_bass_kernel_spmd(
        nc, in_maps, core_ids=list(range(NCORES)), trace=trace
    )
    return _postprocess(res.results), res


def kernel(q, k, v):
    out, _ = run(q, k, v, trace=False)
    return out
